# revision 14
# baseline (speedup 1.0000x reference)
"""Trainium2 Bass kernel for GQA MHA prefill (S=2048, D=4096, H=32, KVH=8).

Strategy (8 NeuronCores, tensor-parallel over heads):
  - Each core owns 4 query heads + 1 kv head. Host stages transposed,
    head-permuted weight shards so no on-chip transposes are needed for
    the projections: qT/kT/vT come out of the PE directly in [dim, seq]
    layout (seq on the free axis).
  - The q/k/v and output projections run as fp8e4 DoubleRow matmuls
    (0.5 PE cycles per output column, two 128-contraction tiles per
    instruction). Accuracy is recovered with a 3-pass hi/lo split:
    W ~ W_hi + W_lo and x ~ x_hi + x_lo (all fp8), accumulating
    W_hi@x_hi + W_lo@x_hi + W_hi@x_lo into PSUM. The dropped lo@lo term
    is O(quant^2); the net matmul error is ~1e-3, below bf16. Weights
    are pre-scaled x64 so their magnitudes sit in e4m3's normal range;
    the 64 is folded back via the RoPE tables (q, k), the softmax
    denominator ones-column (v), and a scaled PSUM->SBUF copy (wo).
  - Head-dim components are permuted (even indices first, odd second) so
    RoPE becomes ops on contiguous partition halves; the permutation is
    applied identically to q and k so logits are unchanged.
  - SDPA stays bf16: logits[k, q] = kT.T @ qT per 128-row k-chunk at
    exact causal granularity (q >= 128*kc); exp on ScalarE; the
    diagonal block is masked multiplicatively on gpsimd after exp.
    Softmax denominators come free as an extra ones-column in the p@v
    matmul. Heads are software-pipelined: logits of head h interleave
    with p@v of head h-1 so the PE never waits on ScalarE's exp.
  - o[q, hd] tiles are normalized, PE-transposed to oT and AllToAll'd
    (4 x 512KB per core) from head-sharded to seq-sharded layout. Each
    core then splits the received rows into hi/lo fp8 on the DVE and
    computes its 256 output rows against the full wo with the same
    3-pass DoubleRow scheme.
  - If the mask input is NOT the expected causal mask, a general
    fallback variant applies the mask as data (identity-matmul
    accumulation into PSUM).
"""

import sys

import numpy as np

sys.path.insert(0, "/opt/trn_rl_repo")

S = 2048
D = 4096
H = 32
KVH = 8
HD = 128
NCORES = 8
HL = H // NCORES          # 4 local query heads
DL = HL * HD              # 512 local q dim
SQ = S // NCORES          # 256 output rows per core
GH = HD // 2              # 64 rope pair lanes
KC = S // 128             # 16 key chunks
DC = D // 128             # 32 contraction chunks
NP = DC // 2              # 16 contraction chunk-pairs
NB = S // 512             # 4 seq blocks of 512
NEG = -1e9
VST = 130                 # v_sb column stride: 128 hd + 1 ones + 1 pad
SW = 64.0                 # fp8 weight pre-scale (folded back downstream)

# causal et strip layout: strip kc covers q in [128*kc, S)
ET_LO = [128 * kc for kc in range(KC)]
ET_W = [S - lo for lo in ET_LO]
ET_OFF = [sum(ET_W[:kc]) for kc in range(KC)]
ET_COLS = sum(ET_W)

# stage-3 contraction order: head-major so each head's AllToAll'd rows
# unlock a quarter of the contraction; DoubleRow pairs are consecutive
# entries (same head, rr stride 2 -> lhsT k-tile stride 2*HL*SQ cols)
CORDER = [rr * HL + h for h in range(HL) for rr in range(NCORES)]

_built = {}


def _build(causal: bool, for_sim: bool = False):
    import concourse.bass as bass  # noqa: F401
    import concourse.mybir as mybir
    import concourse.tile as tile
    from concourse import bacc
    from concourse.masks import make_identity

    fp32 = mybir.dt.float32
    bf16 = mybir.dt.bfloat16
    f8 = mybir.dt.float8e4
    AF = mybir.ActivationFunctionType
    OP = mybir.AluOpType
    DRM = mybir.MatmulPerfMode.DoubleRow

    nc = bacc.Bacc(
        "TRN2",
        target_bir_lowering=False,
        debug=False,
        num_devices=1 if for_sim else NCORES,
    )
    f32r = mybir.dt.float32r
    # x chunk-pairs: [pair, block, part, hi 1024 | lo 1024] fp8
    xT = nc.dram_tensor("xT", [NP, NB, 128, 2048], f8, kind="ExternalInput")
    # wq lhsT image: [128, pair * (hi 4*256 | lo 4*256)] fp8
    wq8 = nc.dram_tensor("wq8", [128, NP * 2048], f8, kind="ExternalInput")
    # wk|wv lhsT image: [128, pair * (kh 256|vh 256|kl 256|vl 256)] fp8
    wkv8 = nc.dram_tensor("wkv8", [128, NP * 1024], f8, kind="ExternalInput")
    cosT = nc.dram_tensor("cosT", [128, S], bf16, kind="ExternalInput")
    sinT = nc.dram_tensor("sinT", [128, S], bf16, kind="ExternalInput")
    # wo rhs image: [pair, group, part, hi 2048 | lo 2048] fp8 (CORDER pairs)
    wo8 = nc.dram_tensor("wo8", [NP, 4, 128, 4096], f8, kind="ExternalInput")
    if not causal:
        maskT = nc.dram_tensor("maskT", [S, S], fp32, kind="ExternalInput")
    out = nc.dram_tensor("out", [SQ, D], bf16, kind="ExternalOutput")

    rg = [list(range(NCORES))]

    with tile.TileContext(nc) as tc:
        with (
            tc.tile_pool(name="const", bufs=1) as constp,
            tc.tile_pool(name="pers", bufs=1) as pers,
            tc.tile_pool(name="dram", bufs=1, space="DRAM") as dramp,
        ):
            ident = constp.tile([128, 128], fp32, tag="ident")
            identb = constp.tile([128, 128], bf16, tag="identb")
            c_sb = constp.tile([128, S], bf16, tag="cos")
            s_sb = constp.tile([128, S], bf16, tag="sin")
            idb = constp.tile([128, 128], bf16, tag="idb")
            psw = constp.tile([128, 128], fp32, tag="psw")
            pswr = constp.tile([128, 128], f32r, tag="pswr")

            def init_constants():
                # deferred so the first weight DMAs aren't queued behind
                # the constant setup on the scalar/gpsimd queues
                make_identity(nc, ident[:])
                nc.scalar.copy(identb[:], ident[:])
                nc.scalar.copy(idb[:], ident[:])
                # half-swap permutation: (Psw^T x)[p] = x[(p+64) % 128]
                nc.gpsimd.memset(psw[:], 0.0)
                for b0 in (64, -64):
                    nc.gpsimd.affine_select(
                        out=psw[:], in_=psw[:],
                        pattern=[[-1, 128]],
                        compare_op=OP.not_equal,
                        fill=1.0,
                        base=b0,
                        channel_multiplier=1,
                    )
                nc.scalar.copy(pswr[:], psw[:])

            qT_sb = pers.tile([128, HL * S], bf16, tag="qT")
            kT_sb = pers.tile([128, S], bf16, tag="kT")
            v_sb = pers.tile([128, KC * VST], bf16, tag="v")
            # et strips live in the persistent pool: allocating them in a
            # stage-2 pool would reuse stage-1's SBUF and serialize the first
            # exp behind stage-1's last consumers (a ~9us pool barrier)
            et_cols = ET_COLS if causal else KC * S
            et_bufs = [
                pers.tile([128, et_cols], bf16, tag=f"et{b}", name=f"et{b}")
                for b in range(2)
            ]

            a2a_in = [
                dramp.tile(
                    [NCORES * HD, SQ], bf16,
                    tag=f"a2a_in{h}", name=f"a2a_in{h}",
                )
                for h in range(HL)
            ]
            a2a_out = [
                dramp.tile(
                    [NCORES * HD, SQ], bf16,
                    tag=f"a2a_out{h}", name=f"a2a_out{h}",
                )
                for h in range(HL)
            ]

            # ---------------- Stage 1: projections + RoPE ----------------
            with (
                tc.tile_pool(name="s1w", bufs=1) as s1w,
                tc.tile_pool(name="s1x", bufs=12) as s1x,
                tc.tile_pool(name="rope", bufs=3) as ropep,
                tc.tile_pool(name="s1v", bufs=3) as s1v,
                tc.tile_pool(name="ps_q", bufs=1, space="PSUM") as ps_q,
                tc.tile_pool(name="ps_kv", bufs=1, space="PSUM") as ps_kv,
                tc.tile_pool(name="ps_tr", bufs=1, space="PSUM") as ps_tr,
                tc.tile_pool(name="ps_sw", bufs=1, space="PSUM") as ps_sw,
            ):
                # lhsT images: per pair p, head m: [128, 2, 128] contiguous
                wq_sb = s1w.tile([128, NP * 2048], f8, tag="wq8")
                wkv_sb = s1w.tile([128, NP * 1024], f8, tag="wkv8")

                def load_kv_quarter(cq):
                    # 4 pairs of [k_hi|v_hi|k_lo|v_lo] (1024 cols each)
                    sl = slice(cq * 4096, (cq + 1) * 4096)
                    nc.sync.dma_start(wkv_sb[:, sl], wkv8[:, sl])

                def load_wq_pair(p, eng=None):  # 256KB: hi+lo of pair p
                    sl = slice(p * 2048, (p + 1) * 2048)
                    (eng or nc.scalar).dma_start(wq_sb[:, sl], wq8[:, sl])

                # first wq pair on the SP queue so its transfer is first in
                # line on the (serial) DMA engines; everything else after
                load_wq_pair(0, eng=nc.sync)
                xt_pre = s1x.tile([128, 2048], f8, tag="xt", name="xtp0")
                nc.sync.dma_start(xt_pre[:], xT[0, 0, :, :])
                load_kv_quarter(0)
                init_constants()
                load_wq_pair(1)
                load_wq_pair(2)

                def rope(dst, stg, col0, ncol):
                    # dst/stg: [128, ncol]; rows 0:64 = even comps, 64:128 odd
                    # dst = stg * cfull + halfswap(stg) * sfull, with
                    # cfull = [c; c] and sfull = [-s; s] staged host-side.
                    swp = ps_sw.tile([128, 512], fp32, tag="swp")
                    nc.tensor.matmul(
                        swp[:, 0:ncol], lhsT=pswr[:], rhs=stg[:, 0:ncol],
                        start=True, stop=True,
                    )
                    t1 = ropep.tile([128, 512], fp32, tag="t1")
                    nc.vector.tensor_tensor(
                        t1[:, 0:ncol], stg[:, 0:ncol].bitcast(fp32),
                        c_sb[:, col0 : col0 + ncol], OP.mult,
                    )
                    t2 = ropep.tile([128, 512], fp32, tag="t2")
                    nc.vector.tensor_tensor(
                        t2[:, 0:ncol], swp[:, 0:ncol],
                        s_sb[:, col0 : col0 + ncol], OP.mult,
                    )
                    nc.vector.tensor_tensor(
                        dst, t1[:, 0:ncol], t2[:, 0:ncol], OP.add
                    )

                for nb in range(NB):
                    qps = [
                        ps_q.tile([128, 512], fp32, tag=f"q{m}", name=f"q{m}")
                        for m in range(HL)
                    ]
                    kps = ps_kv.tile([128, 512], fp32, tag="kk")
                    vps = ps_kv.tile([128, 512], fp32, tag="vv")
                    for p in range(NP):
                        if nb == 0:
                            # stream remaining weight chunks just ahead of
                            # their consumption so they don't head-block xt
                            if 1 <= p <= 13:
                                load_wq_pair(p + 2)
                            if p in (2, 6, 10):
                                load_kv_quarter(p // 4 + 1)
                            if p == 1:
                                nc.sync.dma_start(c_sb[:], cosT[:, :])
                                nc.sync.dma_start(s_sb[:], sinT[:, :])
                        if nb == 0 and p < 1:
                            xt = xt_pre
                        else:
                            xt = s1x.tile([128, 2048], f8, tag="xt")
                            nc.sync.dma_start(xt[:], xT[p, nb, :, :])
                        st = p == 0
                        sp = p == NP - 1
                        xth2 = xt[:, 0:1024].rearrange("q (j m) -> q j m", j=2)
                        xtl2 = xt[:, 1024:2048].rearrange("q (j m) -> q j m", j=2)
                        kvb = p * 1024
                        qb = p * 2048

                        # k, v first: their PSUM banks are released first at
                        # the block boundary (copy order below matches). At
                        # kernel start (nb=0, p=0) q goes first instead: the
                        # wq chunk lands before the wkv chunk.
                        def mm_kv():
                            for off, xt2, s0, s1 in (
                                (0, xth2, st, False),
                                (512, xth2, False, False),
                                (0, xtl2, False, sp),
                            ):
                                nc.tensor.matmul(
                                    kps[:],
                                    lhsT=wkv_sb[
                                        :, kvb + off : kvb + off + 256
                                    ].rearrange("q (j m) -> q j m", j=2),
                                    rhs=xt2,
                                    start=s0,
                                    stop=s1,
                                    perf_mode=DRM,
                                )
                                nc.tensor.matmul(
                                    vps[:],
                                    lhsT=wkv_sb[
                                        :, kvb + off + 256 : kvb + off + 512
                                    ].rearrange("q (j m) -> q j m", j=2),
                                    rhs=xt2,
                                    start=s0,
                                    stop=s1,
                                    perf_mode=DRM,
                                )

                        def mm_q():
                            for off, xt2, s0, s1 in (
                                (0, xth2, st, False),
                                (1024, xth2, False, False),
                                (0, xtl2, False, sp),
                            ):
                                for m in range(HL):
                                    nc.tensor.matmul(
                                        qps[m][:],
                                        lhsT=wq_sb[
                                            :,
                                            qb + off + m * 256 : qb + off + (m + 1) * 256,
                                        ].rearrange("q (j m) -> q j m", j=2),
                                        rhs=xt2,
                                        start=s0,
                                        stop=s1,
                                        perf_mode=DRM,
                                    )

                        if nb == 0 and p == 0:
                            mm_q()
                            mm_kv()
                        else:
                            mm_kv()
                            mm_q()
                    # stage PSUM strips to SBUF (releases the accumulating
                    # banks), then RoPE on DVE from the copies. Copy order
                    # matches next block's matmul order (k, v, q0..q3) and is
                    # split ACT/DVE so the release chain is ~1us, not ~3.4us.
                    stk = s1v.tile([128, 512], f32r, tag="stq", bufs=6)
                    nc.scalar.copy(stk[:], kps[:])
                    if nb == NB - 1:
                        # SDPA is gated on kT: rope k before anything else
                        rope(
                            kT_sb[:, nb * 512 : (nb + 1) * 512],
                            stk[:], nb * 512, 512,
                        )
                    vt = s1v.tile([128, 512], bf16, tag="vt")
                    nc.scalar.copy(vt[:], vps[:])

                    def v_path():
                        # vT psum -> sbuf, then PE-transpose each 128-col
                        # chunk to natural [seq, hd] layout with a ones
                        # column appended (value 64: folds away the x64
                        # weight pre-scale when normalizing by it).
                        for j in range(4):
                            kcg = nb * 4 + j
                            vtp = ps_tr.tile([128, 128], bf16, tag="vtr")
                            nc.tensor.transpose(
                                vtp[:], vt[:, j * 128 : (j + 1) * 128], identb[:]
                            )
                            nc.vector.tensor_copy(
                                v_sb[:, kcg * VST : kcg * VST + 128], vtp[:]
                            )
                            nc.vector.memset(
                                v_sb[:, kcg * VST + 128 : kcg * VST + 129], SW
                            )

                    if nb == NB - 1:
                        # v path ahead of the q ropes: pv(0) and the et/PSUM
                        # reuse barriers depend on it, the q ropes don't
                        v_path()
                    stq = []
                    for m in range(HL):
                        stg = s1v.tile([128, 512], f32r, tag="stq", bufs=6)
                        if m < 2:
                            nc.vector.tensor_copy(stg[:], qps[m][:])
                        else:
                            nc.scalar.copy(stg[:], qps[m][:])
                        stq.append(stg)
                    if nb < NB - 1:
                        rope(
                            kT_sb[:, nb * 512 : (nb + 1) * 512],
                            stk[:], nb * 512, 512,
                        )
                    for m in range(HL):
                        rope(
                            qT_sb[:, m * S + nb * 512 : m * S + (nb + 1) * 512],
                            stq[m],
                            nb * 512,
                            512,
                        )
                    if nb < NB - 1:
                        v_path()

            # ------- Stage 2 + 3: SDPA per head, AllToAll, out-projection -------
            # Stage-3 weight tiles prefetch during SDPA; each head's
            # AllToAll'd rows are split into hi/lo fp8 on the DVE as they
            # land. PSUM: ps_l 2x2 + ps_o 2 + pw 2 = 8 banks.
            with (
                tc.tile_pool(name="wo", bufs=6) as wop,
                tc.tile_pool(name="wolh", bufs=1) as wolh,
                tc.tile_pool(name="lhb", bufs=2) as lhbp,
                tc.tile_pool(name="sd", bufs=2) as sd,
                tc.tile_pool(name="sds", bufs=2) as sds,
                tc.tile_pool(name="msk", bufs=4) as mskp,
            ):
                et3 = sd.tile(
                    [128, et_cols], bf16, tag="et3", bufs=1, name="et3"
                )
                et_rot = [et_bufs[0], et_bufs[1], et3]
                sdpa_ps = tc.tile_pool(name="ps_l", bufs=3, space="PSUM")
                ps_l = sdpa_ps.__enter__()
                sdpa_ps2 = tc.tile_pool(name="ps_o", bufs=2, space="PSUM")
                ps_o = sdpa_ps2.__enter__()

                def emit_logits_causal(h, kc, et, which=None):
                    # fill pl windows covering q in [128*kc, S) exactly.
                    # which=0/1 selects only the near/far 1024-window (used
                    # for head 0 so early exps don't head-block on the last
                    # q-rope of stage 1); None emits all.
                    q_lo = ET_LO[kc]
                    base = ET_OFF[kc] - q_lo
                    for t0 in range((q_lo // 1024) * 1024, S, 1024):
                        if which is not None and t0 // 1024 != which:
                            continue
                        e0 = max(t0, q_lo)
                        pl = ps_l.tile([128, 1024], fp32, tag="pl")
                        q = e0
                        while q < t0 + 1024:
                            w = min(512 - (q % 512), t0 + 1024 - q)
                            nc.tensor.matmul(
                                pl[:, q - t0 : q - t0 + w],
                                lhsT=kT_sb[:, kc * 128 : (kc + 1) * 128],
                                rhs=qT_sb[:, h * S + q : h * S + q + w],
                                start=True,
                                stop=True,
                            )
                            q += w
                        nc.scalar.activation(
                            et[:, base + e0 : base + t0 + 1024],
                            pl[:, e0 - t0 : 1024],
                            AF.Exp,
                        )
                        if t0 <= q_lo:
                            # zero the sub-diagonal half of the leading 128
                            # cols (gpsimd: keeps the DVE softmax chain
                            # unordered w.r.t. exp of the next head)
                            nc.gpsimd.affine_select(
                                out=et[:, base + q_lo : base + q_lo + 128],
                                in_=et[:, base + q_lo : base + q_lo + 128],
                                pattern=[[1, 128]],
                                compare_op=OP.is_ge,
                                fill=0.0,
                                base=0,
                                channel_multiplier=-1,
                            )

                def emit_logits_masked(h, kc, et):
                    # fallback: full q range, additive mask from DRAM
                    for t0 in range(0, S, 1024):
                        pl = ps_l.tile([128, 1024], fp32, tag="pl")
                        for qb in (t0, t0 + 512):
                            fo = qb - t0
                            mt = mskp.tile([128, 512], fp32, tag="mt")
                            nc.sync.dma_start(
                                mt[:],
                                maskT[
                                    kc * 128 : (kc + 1) * 128,
                                    qb : qb + 512,
                                ],
                            )
                            nc.tensor.matmul(
                                pl[:, fo : fo + 512],
                                lhsT=ident[:],
                                rhs=mt[:],
                                start=True,
                                stop=False,
                            )
                            nc.tensor.matmul(
                                pl[:, fo : fo + 512],
                                lhsT=kT_sb[:, kc * 128 : (kc + 1) * 128],
                                rhs=qT_sb[:, h * S + qb : h * S + qb + 512],
                                start=False,
                                stop=True,
                            )
                        nc.scalar.activation(
                            et[:, kc * S + t0 : kc * S + t0 + 1024],
                            pl[:],
                            AF.Exp,
                        )

                osb_q = {}

                def emit_pv_accum(h, qc, et):
                    kc_hi = qc if causal else KC - 1
                    po = ps_o.tile([128, 129], fp32, tag="po")
                    for kc in range(kc_hi + 1):
                        if causal:
                            lo = ET_OFF[kc] - ET_LO[kc] + qc * 128
                        else:
                            lo = kc * S + qc * 128
                        nc.tensor.matmul(
                            po[:],
                            lhsT=et[:, lo : lo + 128],
                            rhs=v_sb[:, kc * VST : kc * VST + 129],
                            start=(kc == 0),
                            stop=(kc == kc_hi),
                        )
                    rc = sds.tile([128, 1], fp32, tag="rc")
                    nc.vector.reciprocal(rc[:], po[:, 128:129])
                    osb = sds.tile([128, 128], bf16, tag="osb", bufs=3)
                    nc.vector.tensor_scalar_mul(osb[:], po[:, 0:128], rc[:])
                    osb_q[(h, qc)] = osb

                def emit_pv_finish(h, qc):
                    osb = osb_q.pop((h, qc))
                    otp = ps_o.tile([128, 258], bf16, tag="po", name="otp")
                    nc.tensor.transpose(otp[:, 0:128], osb[:], idb[:])
                    if qc % 2 == 0:
                        emit_pv_finish.ots = sds.tile(
                            [128, 256], bf16, tag="ots", bufs=4, name="ots"
                        )
                    nc.vector.tensor_copy(
                        emit_pv_finish.ots[:, (qc % 2) * 128 : (qc % 2 + 1) * 128],
                        otp[:, 0:128],
                    )
                    if qc % 2 == 1:
                        nc.sync.dma_start(
                            a2a_in[h][(qc // 2) * 128 : (qc // 2 + 1) * 128, :],
                            emit_pv_finish.ots[:],
                        )

                def emit_collective(h):
                    if for_sim:
                        # timing proxy: collective replaced by local DMA
                        nc.sync.dma_start(a2a_out[h][:], a2a_in[h][:])
                    else:
                        nc.gpsimd.collective_compute(
                            "AllToAll",
                            OP.bypass,
                            replica_groups=rg,
                            ins=[a2a_in[h][:].opt()],
                            outs=[a2a_out[h][:].opt()],
                        )

                # received-rows hi/lo fp8 images, col = rr*HL*SQ + h*SQ + q
                lh_hi = wolh.tile([128, DC * SQ], f8, tag="lhh")
                lh_lo = wolh.tile([128, DC * SQ], f8, tag="lhl")
                lh_hi4 = lh_hi.rearrange("p (rr hh q) -> p rr hh q", rr=NCORES, hh=HL)
                lh_lo4 = lh_lo.rearrange("p (rr hh q) -> p rr hh q", rr=NCORES, hh=HL)
                lh_hir = lh_hi.rearrange("p (rr q) -> p rr q", rr=NCORES)
                lh_lor = lh_lo.rearrange("p (rr q) -> p rr q", rr=NCORES)

                def emit_lh_split(h):
                    # bf16 received rows -> hi/lo fp8 (DVE), per head
                    lhb = lhbp.tile([128, NCORES * SQ], bf16, tag="lhb")
                    lhb3 = lhb.rearrange("p (rr q) -> p rr q", rr=NCORES)
                    nc.sync.dma_start(
                        lhb3[:, :, :],
                        a2a_out[h].rearrange("(rr p) q -> p rr q", p=128),
                    )
                    nc.vector.tensor_copy(lh_hi4[:, :, h, :], lhb3[:, :, :])
                    hib = lhbp.tile([128, NCORES * SQ], bf16, tag="hib")
                    hib3 = hib.rearrange("p (rr q) -> p rr q", rr=NCORES)
                    nc.gpsimd.tensor_copy(hib3[:, :, :], lh_hi4[:, :, h, :])
                    nc.vector.tensor_tensor(
                        lh_lo4[:, :, h, :], lhb3[:, :, :], hib3[:, :, :],
                        OP.subtract,
                    )

                # stage-3 weight prefetch: the first wo pairs, loaded during
                # the last SDPA block so the out-projection never waits
                wt_pre = {}

                def prefetch_wt(j):
                    wt = wop.tile([128, 4096], f8, tag="wt", name=f"wtp{j}")
                    nc.scalar.dma_start(wt[:], wo8[j, 0, :, :])
                    wt_pre[j] = wt

                # head-pipelined SDPA: logits(h) interleave with pv(h-1),
                # with pv shifted 2 iterations later so exp of the new head
                # starts before the big trailing pv groups
                et_prev = None
                for hb in range(HL + 1):
                    et = et_rot[hb % 3] if hb < HL else None
                    for i in range(KC + 3):
                        if hb < HL and i < KC:
                            if not causal:
                                emit_logits_masked(hb, i, et)
                            elif hb == 0:
                                # near windows first: they don't depend on
                                # stage-1's last q-rope block
                                emit_logits_causal(hb, i, et, which=0)
                            else:
                                emit_logits_causal(hb, i, et)
                        if hb > 0 and 2 <= i < KC + 2:
                            emit_pv_accum(hb - 1, i - 2, et_prev)
                        if hb > 0 and i >= 3:
                            emit_pv_finish(hb - 1, i - 3)
                        if hb == 0 and i < 5:
                            prefetch_wt(i)
                    if hb == 0 and causal:
                        for i in range(KC):
                            emit_logits_causal(hb, i, et, which=1)
                    if hb > 0:
                        emit_collective(hb - 1)
                        emit_lh_split(hb - 1)
                    et_prev = et

                sdpa_ps2.__exit__(None, None, None)
                sdpa_ps.__exit__(None, None, None)
                # ------------- Stage 3: output projection -------------
                with (
                    tc.tile_pool(name="woob", bufs=4) as woob,
                    tc.tile_pool(name="ps_w", bufs=2, space="PSUM") as ps_w,
                ):
                    def group_chunks(pw, nbog, j_list):
                        for j in j_list:
                            c0 = CORDER[2 * j]
                            rr0 = c0 // HL
                            h0 = c0 % HL
                            if nbog == 0 and j in wt_pre:
                                wt = wt_pre[j]
                            else:
                                wt = wop.tile([128, 4096], f8, tag="wt")
                                dma_eng = nc.sync if j % 2 == 0 else nc.scalar
                                dma_eng.dma_start(wt[:], wo8[j, nbog, :, :])
                            wth2 = wt[:, 0:2048].rearrange(
                                "p (j m) -> p j m", j=2
                            )
                            wtl2 = wt[:, 2048:4096].rearrange(
                                "p (j m) -> p j m", j=2
                            )
                            # lhsT k-tile pair: rr0 and rr0+1 (CORDER stride)
                            for lh3, wt2, s0, s1 in (
                                (lh_hir, wth2, j == 0, False),
                                (lh_lor, wth2, False, False),
                                (lh_hir, wtl2, False, j == NP - 1),
                            ):
                                for m in range(4):
                                    qb = h0 * SQ + (m % 2) * 128
                                    nc.tensor.matmul(
                                        pw[m][:],
                                        lhsT=lh3[
                                            :, rr0 : rr0 + 2, qb : qb + 128
                                        ],
                                        rhs=wt2[
                                            :, :, (m // 2) * 512 : (m // 2 + 1) * 512
                                        ],
                                        start=s0,
                                        stop=s1,
                                        perf_mode=DRM,
                                    )

                    def group_close(pw, nbog):
                        dma_eng = [nc.gpsimd, nc.sync, nc.scalar, nc.sync]
                        for m in range(4):
                            ob = woob.tile([128, 512], bf16, tag="ob")
                            # 1/SW folds away the x64 wo pre-scale
                            if m % 2 == 0:
                                nc.vector.tensor_scalar_mul(
                                    ob[:], pw[m][:], 1.0 / SW
                                )
                            else:
                                nc.scalar.activation(
                                    ob[:], pw[m][:], AF.Copy, scale=1.0 / SW
                                )
                            dma_eng[m].dma_start(
                                out[
                                    (m % 2) * 128 : (m % 2 + 1) * 128,
                                    (nbog * 2 + m // 2) * 512 : (nbog * 2 + m // 2 + 1) * 512,
                                ],
                                ob[:],
                            )

                    def group_alloc(nbog):
                        return [
                            ps_w.tile(
                                [128, 512], fp32, tag=f"wo{m}",
                                name=f"pw{nbog}_{m}",
                            )
                            for m in range(4)
                        ]

                    for nbog in range(4):
                        pw = group_alloc(nbog)
                        group_chunks(pw, nbog, range(NP))
                        group_close(pw, nbog)
    nc.compile()
    return nc


_PERM = np.concatenate([np.arange(0, HD, 2), np.arange(1, HD, 2)])


def _hilo(a):
    import ml_dtypes

    f8 = ml_dtypes.float8_e4m3
    hi = a.astype(f8)
    lo = (a - hi.astype(np.float32)).astype(f8)
    return hi, lo


def _stage_inputs(x, wq, wk, wv, wo, mask, freqs_cos, freqs_sin, causal):
    alpha = float(HD) ** -0.25  # sqrt of logit scale folded into both ropes
    import ml_dtypes

    bf = ml_dtypes.bfloat16
    # x chunk-pairs: [pair, block, part, hi (j q) | lo (j q)] fp8
    xc = np.ascontiguousarray(
        x.T.reshape(NP, 2, 128, NB, 512).transpose(0, 3, 2, 1, 4)
    ).reshape(NP, NB, 128, 1024)
    xTh, xTl = _hilo(xc)
    xT8 = np.ascontiguousarray(np.concatenate([xTh, xTl], axis=3))
    # wo rhs image: [pair(CORDER), group, part, hi (j c) | lo (j c)], x64
    wot = (wo.T.reshape(DC, 128, 4, 1024) * SW).transpose(0, 2, 1, 3)
    woth, wotl = _hilo(wot)  # [c, g, p, 1024]
    woh = np.ascontiguousarray(
        woth[CORDER].reshape(NP, 2, 4, 128, 1024).transpose(0, 2, 3, 1, 4)
    ).reshape(NP, 4, 128, 2048)
    wol = np.ascontiguousarray(
        wotl[CORDER].reshape(NP, 2, 4, 128, 1024).transpose(0, 2, 3, 1, 4)
    ).reshape(NP, 4, 128, 2048)
    wo8 = np.ascontiguousarray(np.concatenate([woh, wol], axis=3))
    sc = alpha / SW  # fold logit scale + x64 weight pre-scale
    ct = freqs_cos.T * sc
    st = freqs_sin.T * sc
    cosTc = np.ascontiguousarray(np.concatenate([ct, ct], axis=0)).astype(bf)
    sinTc = np.ascontiguousarray(np.concatenate([-st, st], axis=0)).astype(bf)
    if not causal:
        maskTc = np.ascontiguousarray(np.maximum(mask, -60.0).T)
    in_maps = []
    for i in range(NCORES):
        wq_i = (
            wq[i * DL : (i + 1) * DL, :].reshape(HL, HD, D)[:, _PERM, :] * SW
        )
        # lhsT image: [p, (pair m j l)] from wq_i[m*128+l, (2*pair+j)*128+p]
        wq_img = np.ascontiguousarray(
            wq_i.reshape(HL, HD, NP, 2, 128).transpose(4, 2, 0, 3, 1)
        ).reshape(128, NP, HL * 256)
        wqh_i, wql_i = _hilo(wq_img)
        wq8_i = np.ascontiguousarray(
            np.concatenate([wqh_i, wql_i], axis=2)
        ).reshape(128, NP * 2048)
        wk_i = wk[i * HD : (i + 1) * HD, :][_PERM, :] * SW
        wv_i = wv[i * HD : (i + 1) * HD, :] * SW
        # [p, pair, j, l] images for k and v: [k_hi|v_hi|k_lo|v_lo] per pair
        k_img = wk_i.reshape(HD, NP, 2, 128).transpose(3, 1, 2, 0)
        v_img = wv_i.reshape(HD, NP, 2, 128).transpose(3, 1, 2, 0)
        kh, kl = _hilo(k_img.reshape(128, NP, 256))
        vh, vl = _hilo(v_img.reshape(128, NP, 256))
        wkv8_i = np.ascontiguousarray(
            np.concatenate([kh, vh, kl, vl], axis=2)
        ).reshape(128, NP * 1024)
        m = dict(
            xT=xT8,
            wq8=wq8_i,
            wkv8=wkv8_i,
            cosT=cosTc,
            sinT=sinTc,
            wo8=wo8,
        )
        if not causal:
            m["maskT"] = maskTc
        in_maps.append(m)
    return in_maps


def _is_causal(mask):
    if mask.shape != (S, S):
        return False
    tri = np.tril(np.ones((S, S), bool))
    return bool(
        np.all(mask[tri] == 0.0) and np.all(mask[~tri] <= -1e8)
    )


def run(inputs, trace=False):
    from concourse.bass_utils import run_bass_kernel_spmd

    causal = _is_causal(np.asarray(inputs["mask"]))
    if causal not in _built:
        _built[causal] = _build(causal)
    nc = _built[causal]
    in_maps = _stage_inputs(
        np.asarray(inputs["x"], np.float32),
        np.asarray(inputs["wq"], np.float32),
        np.asarray(inputs["wk"], np.float32),
        np.asarray(inputs["wv"], np.float32),
        np.asarray(inputs["wo"], np.float32),
        np.asarray(inputs["mask"], np.float32),
        np.asarray(inputs["freqs_cos"], np.float32),
        np.asarray(inputs["freqs_sin"], np.float32),
        causal,
    )
    res = run_bass_kernel_spmd(
        nc, in_maps, core_ids=list(range(NCORES)), trace=trace
    )
    out = np.concatenate(
        [np.asarray(res.results[i]["out"], np.float32) for i in range(NCORES)],
        axis=0,
    )
    return out, res


def kernel(**inputs):
    out, _ = run(inputs, trace=False)
    return out


# revision 19
# speedup vs baseline: 1.0377x; 1.0377x over previous
"""Trainium2 Bass kernel for GQA MHA prefill (S=2048, D=4096, H=32, KVH=8).

Strategy (8 NeuronCores, tensor-parallel over heads):
  - Each core owns 4 query heads + 1 kv head. Host stages transposed,
    head-permuted weight shards so no on-chip transposes are needed for
    the projections: qT/kT/vT come out of the PE directly in [dim, seq]
    layout (seq on the free axis).
  - The q/k/v and output projections run as fp8e4 DoubleRow matmuls
    (0.5 PE cycles per output column, two 128-contraction tiles per
    instruction). Accuracy is recovered with a 3-pass hi/lo split:
    W ~ W_hi + W_lo and x ~ x_hi + x_lo (all fp8), accumulating
    W_hi@x_hi + W_lo@x_hi + W_hi@x_lo into PSUM. The dropped lo@lo term
    is O(quant^2); the net matmul error is ~1e-3, below bf16. Weights
    are pre-scaled x64 so their magnitudes sit in e4m3's normal range;
    the 64 is folded back via the RoPE tables (q, k), the softmax
    denominator ones-column (v), and a scaled PSUM->SBUF copy (wo).
  - Head-dim components are permuted (even indices first, odd second) so
    RoPE becomes ops on contiguous partition halves; the permutation is
    applied identically to q and k so logits are unchanged.
  - SDPA stays bf16: logits[k, q] = kT.T @ qT per 128-row k-chunk at
    exact causal granularity (q >= 128*kc); exp on ScalarE; the
    diagonal block is masked multiplicatively on gpsimd after exp.
    Softmax denominators come free as an extra ones-column in the p@v
    matmul. Heads are software-pipelined: logits of head h interleave
    with p@v of head h-1 so the PE never waits on ScalarE's exp.
  - o[q, hd] tiles are normalized, PE-transposed to oT and AllToAll'd
    (4 x 512KB per core) from head-sharded to seq-sharded layout. Each
    core then splits the received rows into hi/lo fp8 on the DVE and
    computes its 256 output rows against the full wo with the same
    3-pass DoubleRow scheme.
  - If the mask input is NOT the expected causal mask, a general
    fallback variant applies the mask as data (identity-matmul
    accumulation into PSUM).
"""

import sys

import numpy as np

sys.path.insert(0, "/opt/trn_rl_repo")

S = 2048
D = 4096
H = 32
KVH = 8
HD = 128
NCORES = 8
HL = H // NCORES          # 4 local query heads
DL = HL * HD              # 512 local q dim
SQ = S // NCORES          # 256 output rows per core
GH = HD // 2              # 64 rope pair lanes
KC = S // 128             # 16 key chunks
DC = D // 128             # 32 contraction chunks
NP = DC // 2              # 16 contraction chunk-pairs
NB = S // 512             # 4 seq blocks of 512
NEG = -1e9
VST = 130                 # v_sb column stride: 128 hd + 1 ones + 1 pad
SW = 64.0                 # fp8 weight pre-scale (folded back downstream)

# causal et strip layout: strip kc covers q in [128*kc, S)
ET_LO = [128 * kc for kc in range(KC)]
ET_W = [S - lo for lo in ET_LO]
ET_OFF = [sum(ET_W[:kc]) for kc in range(KC)]
ET_COLS = sum(ET_W)

# stage-3 contraction order: head-major so each head's AllToAll'd rows
# unlock a quarter of the contraction; DoubleRow pairs are consecutive
# entries (same head, rr stride 2 -> lhsT k-tile stride 2*HL*SQ cols)
CORDER = [rr * HL + h for h in range(HL) for rr in range(NCORES)]

_built = {}


def _build(causal: bool, for_sim: bool = False):
    import concourse.bass as bass  # noqa: F401
    import concourse.mybir as mybir
    import concourse.tile as tile
    from concourse import bacc
    from concourse.masks import make_identity

    fp32 = mybir.dt.float32
    bf16 = mybir.dt.bfloat16
    f8 = mybir.dt.float8e4
    AF = mybir.ActivationFunctionType
    OP = mybir.AluOpType
    DRM = mybir.MatmulPerfMode.DoubleRow

    nc = bacc.Bacc(
        "TRN2",
        target_bir_lowering=False,
        debug=False,
        num_devices=1 if for_sim else NCORES,
    )
    f32r = mybir.dt.float32r
    # x chunk-pairs: [pair, block, part, hi 1024 | lo 1024] fp8
    xT = nc.dram_tensor("xT", [NP, NB, 128, 2048], f8, kind="ExternalInput")
    # wq lhsT image: [128, pair * (hi 4*256 | lo 4*256)] fp8
    wq8 = nc.dram_tensor("wq8", [128, NP * 2048], f8, kind="ExternalInput")
    # wk|wv lhsT image: [128, pair * (kh 256|vh 256|kl 256|vl 256)] fp8
    wkv8 = nc.dram_tensor("wkv8", [128, NP * 1024], f8, kind="ExternalInput")
    cosT = nc.dram_tensor("cosT", [128, S], bf16, kind="ExternalInput")
    sinT = nc.dram_tensor("sinT", [128, S], bf16, kind="ExternalInput")
    # wo rhs image: [pair, group, part, hi 2048 | lo 2048] fp8 (CORDER pairs)
    wo8 = nc.dram_tensor("wo8", [NP, 4, 128, 4096], f8, kind="ExternalInput")
    if not causal:
        maskT = nc.dram_tensor("maskT", [S, S], fp32, kind="ExternalInput")
    out = nc.dram_tensor("out", [SQ, D], bf16, kind="ExternalOutput")

    rg = [list(range(NCORES))]

    with tile.TileContext(nc) as tc:
        with (
            tc.tile_pool(name="const", bufs=1) as constp,
            tc.tile_pool(name="pers", bufs=1) as pers,
            tc.tile_pool(name="dram", bufs=1, space="DRAM") as dramp,
        ):
            ident = constp.tile([128, 128], fp32, tag="ident")
            identb = constp.tile([128, 128], bf16, tag="identb")
            c_sb = constp.tile([128, S], bf16, tag="cos")
            s_sb = constp.tile([128, S], bf16, tag="sin")
            idb = constp.tile([128, 128], bf16, tag="idb")
            psw = constp.tile([128, 128], fp32, tag="psw")
            pswr = constp.tile([128, 128], f32r, tag="pswr")

            def init_constants():
                # deferred so the first weight DMAs aren't queued behind
                # the constant setup on the scalar/gpsimd queues
                make_identity(nc, ident[:])
                nc.scalar.copy(identb[:], ident[:])
                nc.scalar.copy(idb[:], ident[:])
                # half-swap permutation: (Psw^T x)[p] = x[(p+64) % 128]
                nc.gpsimd.memset(psw[:], 0.0)
                for b0 in (64, -64):
                    nc.gpsimd.affine_select(
                        out=psw[:], in_=psw[:],
                        pattern=[[-1, 128]],
                        compare_op=OP.not_equal,
                        fill=1.0,
                        base=b0,
                        channel_multiplier=1,
                    )
                nc.scalar.copy(pswr[:], psw[:])

            qT_sb = pers.tile([128, HL * S], bf16, tag="qT")
            kT_sb = pers.tile([128, S], bf16, tag="kT")
            v_sb = pers.tile([128, KC * VST], bf16, tag="v")
            # et strips live in the persistent pool: allocating them in a
            # stage-2 pool would reuse stage-1's SBUF and serialize the first
            # exp behind stage-1's last consumers (a ~9us pool barrier)
            et_cols = ET_COLS if causal else KC * S
            et_bufs = [
                pers.tile([128, et_cols], bf16, tag=f"et{b}", name=f"et{b}")
                for b in range(2)
            ]

            a2a_in = [
                dramp.tile(
                    [NCORES * HD, SQ], bf16,
                    tag=f"a2a_in{h}", name=f"a2a_in{h}",
                )
                for h in range(HL)
            ]
            a2a_out = [
                dramp.tile(
                    [NCORES * HD, SQ], bf16,
                    tag=f"a2a_out{h}", name=f"a2a_out{h}",
                )
                for h in range(HL)
            ]

            # ---------------- Stage 1: projections + RoPE ----------------
            with (
                tc.tile_pool(name="s1w", bufs=1) as s1w,
                tc.tile_pool(name="s1x", bufs=12) as s1x,
                tc.tile_pool(name="rope", bufs=3) as ropep,
                tc.tile_pool(name="s1v", bufs=3) as s1v,
                tc.tile_pool(name="ps_q", bufs=1, space="PSUM") as ps_q,
                tc.tile_pool(name="ps_kv", bufs=1, space="PSUM") as ps_kv,
                tc.tile_pool(name="ps_tr", bufs=1, space="PSUM") as ps_tr,
                tc.tile_pool(name="ps_sw", bufs=1, space="PSUM") as ps_sw,
            ):
                # lhsT images: per pair p, head m: [128, 2, 128] contiguous
                wq_sb = s1w.tile([128, NP * 2048], f8, tag="wq8")
                wkv_sb = s1w.tile([128, NP * 1024], f8, tag="wkv8")

                def load_kv_quarter(cq):
                    # 4 pairs of [k_hi|v_hi|k_lo|v_lo] (1024 cols each)
                    sl = slice(cq * 4096, (cq + 1) * 4096)
                    nc.sync.dma_start(wkv_sb[:, sl], wkv8[:, sl])

                def load_wq_pair(p, eng=None):  # 256KB: hi+lo of pair p
                    sl = slice(p * 2048, (p + 1) * 2048)
                    (eng or nc.scalar).dma_start(wq_sb[:, sl], wq8[:, sl])

                # first wq pair on the SP queue so its transfer is first in
                # line on the (serial) DMA engines; everything else after
                load_wq_pair(0, eng=nc.sync)
                xt_pre = []
                for pp in range(2):
                    xtp = s1x.tile([128, 2048], f8, tag="xt", name=f"xtp{pp}")
                    nc.sync.dma_start(xtp[:], xT[pp, 0, :, :])
                    xt_pre.append(xtp)
                load_kv_quarter(0)
                init_constants()
                load_wq_pair(1)
                load_wq_pair(2)

                def rope(dst, stg, col0, ncol):
                    # dst/stg: [128, ncol]; rows 0:64 = even comps, 64:128 odd
                    # dst = stg * cfull + halfswap(stg) * sfull, with
                    # cfull = [c; c] and sfull = [-s; s] staged host-side.
                    swp = ps_sw.tile([128, 512], fp32, tag="swp")
                    nc.tensor.matmul(
                        swp[:, 0:ncol], lhsT=pswr[:], rhs=stg[:, 0:ncol],
                        start=True, stop=True,
                    )
                    t1 = ropep.tile([128, 512], fp32, tag="t1")
                    nc.vector.tensor_tensor(
                        t1[:, 0:ncol], stg[:, 0:ncol].bitcast(fp32),
                        c_sb[:, col0 : col0 + ncol], OP.mult,
                    )
                    t2 = ropep.tile([128, 512], fp32, tag="t2")
                    nc.vector.tensor_tensor(
                        t2[:, 0:ncol], swp[:, 0:ncol],
                        s_sb[:, col0 : col0 + ncol], OP.mult,
                    )
                    nc.vector.tensor_tensor(
                        dst, t1[:, 0:ncol], t2[:, 0:ncol], OP.add
                    )

                for nb in range(NB):
                    qps = [
                        ps_q.tile([128, 512], fp32, tag=f"q{m}", name=f"q{m}")
                        for m in range(HL)
                    ]
                    kps = ps_kv.tile([128, 512], fp32, tag="kk")
                    vps = ps_kv.tile([128, 512], fp32, tag="vv")
                    for p in range(NP):
                        if nb == 0:
                            # stream remaining weight chunks just ahead of
                            # their consumption so they don't head-block xt
                            if 1 <= p <= 13:
                                load_wq_pair(p + 2)
                            if p in (2, 6, 10):
                                load_kv_quarter(p // 4 + 1)
                            if p == 8:
                                nc.sync.dma_start(c_sb[:], cosT[:, :])
                                nc.sync.dma_start(s_sb[:], sinT[:, :])
                        if nb == 0 and p < 2:
                            xt = xt_pre[p]
                        else:
                            xt = s1x.tile([128, 2048], f8, tag="xt")
                            nc.sync.dma_start(xt[:], xT[p, nb, :, :])
                        st = p == 0
                        sp = p == NP - 1
                        xth2 = xt[:, 0:1024].rearrange("q (j m) -> q j m", j=2)
                        xtl2 = xt[:, 1024:2048].rearrange("q (j m) -> q j m", j=2)
                        kvb = p * 1024
                        qb = p * 2048

                        # k, v first: their PSUM banks are released first at
                        # the block boundary (copy order below matches). At
                        # kernel start (nb=0, p=0) q goes first instead: the
                        # wq chunk lands before the wkv chunk.
                        def mm_kv():
                            for off, xt2, s0, s1 in (
                                (0, xth2, st, False),
                                (512, xth2, False, False),
                                (0, xtl2, False, sp),
                            ):
                                nc.tensor.matmul(
                                    kps[:],
                                    lhsT=wkv_sb[
                                        :, kvb + off : kvb + off + 256
                                    ].rearrange("q (j m) -> q j m", j=2),
                                    rhs=xt2,
                                    start=s0,
                                    stop=s1,
                                    perf_mode=DRM,
                                )
                                nc.tensor.matmul(
                                    vps[:],
                                    lhsT=wkv_sb[
                                        :, kvb + off + 256 : kvb + off + 512
                                    ].rearrange("q (j m) -> q j m", j=2),
                                    rhs=xt2,
                                    start=s0,
                                    stop=s1,
                                    perf_mode=DRM,
                                )

                        def mm_q():
                            for off, xt2, s0, s1 in (
                                (0, xth2, st, False),
                                (1024, xth2, False, False),
                                (0, xtl2, False, sp),
                            ):
                                for m in range(HL):
                                    nc.tensor.matmul(
                                        qps[m][:],
                                        lhsT=wq_sb[
                                            :,
                                            qb + off + m * 256 : qb + off + (m + 1) * 256,
                                        ].rearrange("q (j m) -> q j m", j=2),
                                        rhs=xt2,
                                        start=s0,
                                        stop=s1,
                                        perf_mode=DRM,
                                    )

                        if nb == 0 and p == 0:
                            mm_q()
                            mm_kv()
                        else:
                            mm_kv()
                            mm_q()
                    # stage PSUM strips to SBUF (releases the accumulating
                    # banks), then RoPE on DVE from the copies. Copy order
                    # matches next block's matmul order (k, v, q0..q3) and is
                    # split ACT/DVE so the release chain is ~1us, not ~3.4us.
                    stk = s1v.tile([128, 512], f32r, tag="stq", bufs=6)
                    nc.scalar.copy(stk[:], kps[:])
                    if nb == NB - 1:
                        # SDPA is gated on kT: rope k before anything else
                        rope(
                            kT_sb[:, nb * 512 : (nb + 1) * 512],
                            stk[:], nb * 512, 512,
                        )
                    vt = s1v.tile([128, 512], bf16, tag="vt")
                    nc.scalar.copy(vt[:], vps[:])

                    def v_path():
                        # vT psum -> sbuf, then PE-transpose each 128-col
                        # chunk to natural [seq, hd] layout with a ones
                        # column appended (value 64: folds away the x64
                        # weight pre-scale when normalizing by it).
                        for j in range(4):
                            kcg = nb * 4 + j
                            vtp = ps_tr.tile([128, 128], bf16, tag="vtr")
                            nc.tensor.transpose(
                                vtp[:], vt[:, j * 128 : (j + 1) * 128], identb[:]
                            )
                            nc.vector.tensor_copy(
                                v_sb[:, kcg * VST : kcg * VST + 128], vtp[:]
                            )
                            nc.vector.memset(
                                v_sb[:, kcg * VST + 128 : kcg * VST + 129], SW
                            )

                    if nb == NB - 1:
                        # v path ahead of the q ropes: pv(0) and the et/PSUM
                        # reuse barriers depend on it, the q ropes don't
                        v_path()
                    stq = []
                    for m in range(HL):
                        stg = s1v.tile([128, 512], f32r, tag="stq", bufs=6)
                        if m < 2:
                            nc.vector.tensor_copy(stg[:], qps[m][:])
                        else:
                            nc.scalar.copy(stg[:], qps[m][:])
                        stq.append(stg)
                    if nb < NB - 1:
                        rope(
                            kT_sb[:, nb * 512 : (nb + 1) * 512],
                            stk[:], nb * 512, 512,
                        )
                    for m in range(HL):
                        rope(
                            qT_sb[:, m * S + nb * 512 : m * S + (nb + 1) * 512],
                            stq[m],
                            nb * 512,
                            512,
                        )
                    if nb < NB - 1:
                        v_path()

            # ------- Stage 2 + 3: SDPA per head, AllToAll, out-projection -------
            # Stage-3 weight tiles prefetch during SDPA; each head's
            # AllToAll'd rows are split into hi/lo fp8 on the DVE as they
            # land. PSUM: ps_l 2x2 + ps_o 2 + pw 2 = 8 banks.
            with (
                tc.tile_pool(name="wo", bufs=8) as wop,
                tc.tile_pool(name="wolh", bufs=1) as wolh,
                tc.tile_pool(name="lhb", bufs=2) as lhbp,
                tc.tile_pool(name="sd", bufs=2) as sd,
                tc.tile_pool(name="sds", bufs=2) as sds,
                tc.tile_pool(name="msk", bufs=4) as mskp,
            ):
                et3 = sd.tile(
                    [128, et_cols], bf16, tag="et3", bufs=1, name="et3"
                )
                et_rot = [et_bufs[0], et_bufs[1], et3]
                sdpa_ps = tc.tile_pool(name="ps_l", bufs=3, space="PSUM")
                ps_l = sdpa_ps.__enter__()
                sdpa_ps2 = tc.tile_pool(name="ps_o", bufs=2, space="PSUM")
                ps_o = sdpa_ps2.__enter__()

                def emit_logits_causal(h, kc, et, which=None):
                    # fill pl windows covering q in [128*kc, S) exactly.
                    # which=0/1 selects only the near/far 1024-window (used
                    # for head 0 so early exps don't head-block on the last
                    # q-rope of stage 1); None emits all.
                    q_lo = ET_LO[kc]
                    base = ET_OFF[kc] - q_lo
                    for t0 in range((q_lo // 1024) * 1024, S, 1024):
                        if which is not None and t0 // 1024 != which:
                            continue
                        e0 = max(t0, q_lo)
                        pl = ps_l.tile([128, 1024], fp32, tag="pl")
                        q = e0
                        while q < t0 + 1024:
                            w = min(512 - (q % 512), t0 + 1024 - q)
                            nc.tensor.matmul(
                                pl[:, q - t0 : q - t0 + w],
                                lhsT=kT_sb[:, kc * 128 : (kc + 1) * 128],
                                rhs=qT_sb[:, h * S + q : h * S + q + w],
                                start=True,
                                stop=True,
                            )
                            q += w
                        nc.scalar.activation(
                            et[:, base + e0 : base + t0 + 1024],
                            pl[:, e0 - t0 : 1024],
                            AF.Exp,
                        )
                        if t0 <= q_lo:
                            # zero the sub-diagonal half of the leading 128
                            # cols (gpsimd: keeps the DVE softmax chain
                            # unordered w.r.t. exp of the next head)
                            nc.gpsimd.affine_select(
                                out=et[:, base + q_lo : base + q_lo + 128],
                                in_=et[:, base + q_lo : base + q_lo + 128],
                                pattern=[[1, 128]],
                                compare_op=OP.is_ge,
                                fill=0.0,
                                base=0,
                                channel_multiplier=-1,
                            )

                def emit_logits_masked(h, kc, et):
                    # fallback: full q range, additive mask from DRAM
                    for t0 in range(0, S, 1024):
                        pl = ps_l.tile([128, 1024], fp32, tag="pl")
                        for qb in (t0, t0 + 512):
                            fo = qb - t0
                            mt = mskp.tile([128, 512], fp32, tag="mt")
                            nc.sync.dma_start(
                                mt[:],
                                maskT[
                                    kc * 128 : (kc + 1) * 128,
                                    qb : qb + 512,
                                ],
                            )
                            nc.tensor.matmul(
                                pl[:, fo : fo + 512],
                                lhsT=ident[:],
                                rhs=mt[:],
                                start=True,
                                stop=False,
                            )
                            nc.tensor.matmul(
                                pl[:, fo : fo + 512],
                                lhsT=kT_sb[:, kc * 128 : (kc + 1) * 128],
                                rhs=qT_sb[:, h * S + qb : h * S + qb + 512],
                                start=False,
                                stop=True,
                            )
                        nc.scalar.activation(
                            et[:, kc * S + t0 : kc * S + t0 + 1024],
                            pl[:],
                            AF.Exp,
                        )

                osb_q = {}

                def emit_pv_accum(h, qc, et):
                    kc_hi = qc if causal else KC - 1
                    po = ps_o.tile([128, 129], fp32, tag="po")
                    for kc in range(kc_hi + 1):
                        if causal:
                            lo = ET_OFF[kc] - ET_LO[kc] + qc * 128
                        else:
                            lo = kc * S + qc * 128
                        nc.tensor.matmul(
                            po[:],
                            lhsT=et[:, lo : lo + 128],
                            rhs=v_sb[:, kc * VST : kc * VST + 129],
                            start=(kc == 0),
                            stop=(kc == kc_hi),
                        )
                    rc = sds.tile([128, 1], fp32, tag="rc")
                    nc.vector.reciprocal(rc[:], po[:, 128:129])
                    osb = sds.tile([128, 128], bf16, tag="osb", bufs=3)
                    nc.vector.tensor_scalar_mul(osb[:], po[:, 0:128], rc[:])
                    osb_q[(h, qc)] = osb

                def emit_pv_finish(h, qc):
                    osb = osb_q.pop((h, qc))
                    otp = ps_o.tile([128, 258], bf16, tag="po", name="otp")
                    nc.tensor.transpose(otp[:, 0:128], osb[:], idb[:])
                    if qc % 2 == 0:
                        emit_pv_finish.ots = sds.tile(
                            [128, 256], bf16, tag="ots", bufs=4, name="ots"
                        )
                    nc.vector.tensor_copy(
                        emit_pv_finish.ots[:, (qc % 2) * 128 : (qc % 2 + 1) * 128],
                        otp[:, 0:128],
                    )
                    if qc % 2 == 1:
                        nc.sync.dma_start(
                            a2a_in[h][(qc // 2) * 128 : (qc // 2 + 1) * 128, :],
                            emit_pv_finish.ots[:],
                        )

                def emit_collective(h):
                    if for_sim:
                        # timing proxy: collective replaced by local DMA
                        nc.sync.dma_start(a2a_out[h][:], a2a_in[h][:])
                    else:
                        nc.gpsimd.collective_compute(
                            "AllToAll",
                            OP.bypass,
                            replica_groups=rg,
                            ins=[a2a_in[h][:].opt()],
                            outs=[a2a_out[h][:].opt()],
                        )

                # received-rows hi/lo fp8 images, col = rr*HL*SQ + h*SQ + q
                lh_hi = wolh.tile([128, DC * SQ], f8, tag="lhh")
                lh_lo = wolh.tile([128, DC * SQ], f8, tag="lhl")
                lh_hi4 = lh_hi.rearrange("p (rr hh q) -> p rr hh q", rr=NCORES, hh=HL)
                lh_lo4 = lh_lo.rearrange("p (rr hh q) -> p rr hh q", rr=NCORES, hh=HL)
                lh_hir = lh_hi.rearrange("p (rr q) -> p rr q", rr=NCORES)
                lh_lor = lh_lo.rearrange("p (rr q) -> p rr q", rr=NCORES)

                def emit_lh_split(h):
                    # bf16 received rows -> hi/lo fp8 (DVE), per head
                    lhb = lhbp.tile([128, NCORES * SQ], bf16, tag="lhb")
                    lhb3 = lhb.rearrange("p (rr q) -> p rr q", rr=NCORES)
                    nc.sync.dma_start(
                        lhb3[:, :, :],
                        a2a_out[h].rearrange("(rr p) q -> p rr q", p=128),
                    )
                    nc.vector.tensor_copy(lh_hi4[:, :, h, :], lhb3[:, :, :])
                    hib = lhbp.tile([128, NCORES * SQ], bf16, tag="hib")
                    hib3 = hib.rearrange("p (rr q) -> p rr q", rr=NCORES)
                    nc.gpsimd.tensor_copy(hib3[:, :, :], lh_hi4[:, :, h, :])
                    nc.vector.tensor_tensor(
                        lh_lo4[:, :, h, :], lhb3[:, :, :], hib3[:, :, :],
                        OP.subtract,
                    )

                # stage-3 weight prefetch: the first wo pairs, loaded during
                # the last SDPA block so the out-projection never waits
                wt_pre = {}

                def prefetch_wt(j):
                    wt = wop.tile([128, 4096], f8, tag="wt", name=f"wtp{j}")
                    nc.gpsimd.dma_start(wt[:], wo8[j, 0, :, :])
                    wt_pre[j] = wt

                # head-pipelined SDPA: logits(h) interleave with pv(h-1),
                # with pv shifted 2 iterations later so exp of the new head
                # starts before the big trailing pv groups
                et_prev = None
                for hb in range(HL + 1):
                    et = et_rot[hb % 3] if hb < HL else None
                    for i in range(KC + 3):
                        if hb < HL and i < KC:
                            if not causal:
                                emit_logits_masked(hb, i, et)
                            elif hb == 0:
                                # near windows first: they don't depend on
                                # stage-1's last q-rope block
                                emit_logits_causal(hb, i, et, which=0)
                            else:
                                emit_logits_causal(hb, i, et)
                        if hb > 0 and 2 <= i < KC + 2:
                            emit_pv_accum(hb - 1, i - 2, et_prev)
                        if hb > 0 and i >= 3:
                            emit_pv_finish(hb - 1, i - 3)
                        if hb == 0 and i < 8:
                            prefetch_wt(i)
                    if hb == 0 and causal:
                        for i in range(KC):
                            emit_logits_causal(hb, i, et, which=1)
                    if hb > 0:
                        emit_collective(hb - 1)
                        emit_lh_split(hb - 1)
                    et_prev = et

                sdpa_ps2.__exit__(None, None, None)
                sdpa_ps.__exit__(None, None, None)
                # ------------- Stage 3: output projection -------------
                with (
                    tc.tile_pool(name="woob", bufs=4) as woob,
                    tc.tile_pool(name="ps_w", bufs=2, space="PSUM") as ps_w,
                ):
                    def group_chunks(pw, nbog, j_list):
                        for j in j_list:
                            c0 = CORDER[2 * j]
                            rr0 = c0 // HL
                            h0 = c0 % HL
                            if nbog == 0 and j in wt_pre:
                                wt = wt_pre[j]
                            else:
                                wt = wop.tile([128, 4096], f8, tag="wt")
                                dma_eng = nc.sync if j % 2 == 0 else nc.scalar
                                dma_eng.dma_start(wt[:], wo8[j, nbog, :, :])
                            wth2 = wt[:, 0:2048].rearrange(
                                "p (j m) -> p j m", j=2
                            )
                            wtl2 = wt[:, 2048:4096].rearrange(
                                "p (j m) -> p j m", j=2
                            )
                            # lhsT k-tile pair: rr0 and rr0+1 (CORDER stride)
                            for lh3, wt2, s0, s1 in (
                                (lh_hir, wth2, j == 0, False),
                                (lh_lor, wth2, False, False),
                                (lh_hir, wtl2, False, j == NP - 1),
                            ):
                                for m in range(4):
                                    qb = h0 * SQ + (m % 2) * 128
                                    nc.tensor.matmul(
                                        pw[m][:],
                                        lhsT=lh3[
                                            :, rr0 : rr0 + 2, qb : qb + 128
                                        ],
                                        rhs=wt2[
                                            :, :, (m // 2) * 512 : (m // 2 + 1) * 512
                                        ],
                                        start=s0,
                                        stop=s1,
                                        perf_mode=DRM,
                                    )

                    def group_close(pw, nbog):
                        dma_eng = [nc.gpsimd, nc.sync, nc.scalar, nc.sync]
                        for m in range(4):
                            ob = woob.tile([128, 512], bf16, tag="ob")
                            # 1/SW folds away the x64 wo pre-scale
                            if m % 2 == 0:
                                nc.vector.tensor_scalar_mul(
                                    ob[:], pw[m][:], 1.0 / SW
                                )
                            else:
                                nc.scalar.activation(
                                    ob[:], pw[m][:], AF.Copy, scale=1.0 / SW
                                )
                            dma_eng[m].dma_start(
                                out[
                                    (m % 2) * 128 : (m % 2 + 1) * 128,
                                    (nbog * 2 + m // 2) * 512 : (nbog * 2 + m // 2 + 1) * 512,
                                ],
                                ob[:],
                            )

                    def group_alloc(nbog):
                        return [
                            ps_w.tile(
                                [128, 512], fp32, tag=f"wo{m}",
                                name=f"pw{nbog}_{m}",
                            )
                            for m in range(4)
                        ]

                    for nbog in range(4):
                        pw = group_alloc(nbog)
                        group_chunks(pw, nbog, range(NP))
                        group_close(pw, nbog)
    nc.compile()
    return nc


_PERM = np.concatenate([np.arange(0, HD, 2), np.arange(1, HD, 2)])


def _hilo(a):
    import ml_dtypes

    f8 = ml_dtypes.float8_e4m3
    hi = a.astype(f8)
    lo = (a - hi.astype(np.float32)).astype(f8)
    return hi, lo


def _stage_inputs(x, wq, wk, wv, wo, mask, freqs_cos, freqs_sin, causal):
    alpha = float(HD) ** -0.25  # sqrt of logit scale folded into both ropes
    import ml_dtypes

    bf = ml_dtypes.bfloat16
    # x chunk-pairs: [pair, block, part, hi (j q) | lo (j q)] fp8
    xc = np.ascontiguousarray(
        x.T.reshape(NP, 2, 128, NB, 512).transpose(0, 3, 2, 1, 4)
    ).reshape(NP, NB, 128, 1024)
    xTh, xTl = _hilo(xc)
    xT8 = np.ascontiguousarray(np.concatenate([xTh, xTl], axis=3))
    # wo rhs image: [pair(CORDER), group, part, hi (j c) | lo (j c)], x64
    wot = (wo.T.reshape(DC, 128, 4, 1024) * SW).transpose(0, 2, 1, 3)
    woth, wotl = _hilo(wot)  # [c, g, p, 1024]
    woh = np.ascontiguousarray(
        woth[CORDER].reshape(NP, 2, 4, 128, 1024).transpose(0, 2, 3, 1, 4)
    ).reshape(NP, 4, 128, 2048)
    wol = np.ascontiguousarray(
        wotl[CORDER].reshape(NP, 2, 4, 128, 1024).transpose(0, 2, 3, 1, 4)
    ).reshape(NP, 4, 128, 2048)
    wo8 = np.ascontiguousarray(np.concatenate([woh, wol], axis=3))
    sc = alpha / SW  # fold logit scale + x64 weight pre-scale
    ct = freqs_cos.T * sc
    st = freqs_sin.T * sc
    cosTc = np.ascontiguousarray(np.concatenate([ct, ct], axis=0)).astype(bf)
    sinTc = np.ascontiguousarray(np.concatenate([-st, st], axis=0)).astype(bf)
    if not causal:
        maskTc = np.ascontiguousarray(np.maximum(mask, -60.0).T)
    in_maps = []
    for i in range(NCORES):
        wq_i = (
            wq[i * DL : (i + 1) * DL, :].reshape(HL, HD, D)[:, _PERM, :] * SW
        )
        # lhsT image: [p, (pair m j l)] from wq_i[m*128+l, (2*pair+j)*128+p]
        wq_img = np.ascontiguousarray(
            wq_i.reshape(HL, HD, NP, 2, 128).transpose(4, 2, 0, 3, 1)
        ).reshape(128, NP, HL * 256)
        wqh_i, wql_i = _hilo(wq_img)
        wq8_i = np.ascontiguousarray(
            np.concatenate([wqh_i, wql_i], axis=2)
        ).reshape(128, NP * 2048)
        wk_i = wk[i * HD : (i + 1) * HD, :][_PERM, :] * SW
        wv_i = wv[i * HD : (i + 1) * HD, :] * SW
        # [p, pair, j, l] images for k and v: [k_hi|v_hi|k_lo|v_lo] per pair
        k_img = wk_i.reshape(HD, NP, 2, 128).transpose(3, 1, 2, 0)
        v_img = wv_i.reshape(HD, NP, 2, 128).transpose(3, 1, 2, 0)
        kh, kl = _hilo(k_img.reshape(128, NP, 256))
        vh, vl = _hilo(v_img.reshape(128, NP, 256))
        wkv8_i = np.ascontiguousarray(
            np.concatenate([kh, vh, kl, vl], axis=2)
        ).reshape(128, NP * 1024)
        m = dict(
            xT=xT8,
            wq8=wq8_i,
            wkv8=wkv8_i,
            cosT=cosTc,
            sinT=sinTc,
            wo8=wo8,
        )
        if not causal:
            m["maskT"] = maskTc
        in_maps.append(m)
    return in_maps


def _is_causal(mask):
    if mask.shape != (S, S):
        return False
    tri = np.tril(np.ones((S, S), bool))
    return bool(
        np.all(mask[tri] == 0.0) and np.all(mask[~tri] <= -1e8)
    )


def run(inputs, trace=False):
    from concourse.bass_utils import run_bass_kernel_spmd

    causal = _is_causal(np.asarray(inputs["mask"]))
    if causal not in _built:
        _built[causal] = _build(causal)
    nc = _built[causal]
    in_maps = _stage_inputs(
        np.asarray(inputs["x"], np.float32),
        np.asarray(inputs["wq"], np.float32),
        np.asarray(inputs["wk"], np.float32),
        np.asarray(inputs["wv"], np.float32),
        np.asarray(inputs["wo"], np.float32),
        np.asarray(inputs["mask"], np.float32),
        np.asarray(inputs["freqs_cos"], np.float32),
        np.asarray(inputs["freqs_sin"], np.float32),
        causal,
    )
    res = run_bass_kernel_spmd(
        nc, in_maps, core_ids=list(range(NCORES)), trace=trace
    )
    out = np.concatenate(
        [np.asarray(res.results[i]["out"], np.float32) for i in range(NCORES)],
        axis=0,
    )
    return out, res


def kernel(**inputs):
    out, _ = run(inputs, trace=False)
    return out
